# revision 12
# baseline (speedup 1.0000x reference)
"""Trainium2 Bass kernel for nn_CondenseSFR (BN+ReLU+shuffle+grouped1x1conv+reindex).

Algebra: out = einsum('nchw,cd->ndhw', conv(shuffle(relu(bn(x)))), index).
Everything except the ReLU is linear in the channel dimension, and the BN
scale inv = gamma*rsqrt(var+eps) is strictly positive, so
    relu(inv*x + b) = inv * relu(x + b/inv)
and the shuffle + grouped conv + reindex fold into a single dense 512x512
channel matrix applied after the ReLU:
    out[n,d,s] = sum_c B[d,c] * relu(x[n,c,s] + bprime[c])
with B = (index^T @ A) * inv[None,:],  A the shuffle-permuted block-diagonal
conv weight, bprime = (beta - mean*inv)/inv.

Tolerance is 2e-2 so the whole pipeline runs in bf16 (measured end-to-end
rel err ~3.3e-3): x, bias, weights and the stored output are bf16; matmul
accumulation stays fp32 in PSUM. vs the fp32 version this halves HBM
traffic (17.8 -> 8.9 MB/core, the fp32 kernel sat at the ~358 GB/s
HBM-per-core wall) and makes the PE the critical path (~27.5us of N=512
bf16 matmul streaming per core).

Device work per core (4 of 32 images, data-parallel over 8 cores):
  - x image loads as 4 partition-tiles [128, 1024] bf16; the per-channel
    ReLU bias rides in a separate tiny fp32 [128, CT] tensor loaded once
    up front (tensor_scalar's vector operand must be f32)
  - input chunks alternate the two HWDGE rings (Sync+Scalar); outputs
    split between the SWDGE queue and the HWDGE rings as they drain
  - VectorE: relu(x + bias_c) as one fused tensor_scalar(add,max) in
    bf16 (4x packed mode)
  - TensorE: ct-major PSUM accumulation - 8 banks hold all (d-tile, half)
    outputs of one image, so matmuls start when the first channel tile
    lands; bf16 weights get the automatic FWL fast weight load
  - PSUM -> SBUF evacuation casts fp32 -> bf16 (ACT engine, DVE helps on
    the last image) so stores are half-width too
"""

import numpy as np

import concourse.bacc as bacc
import concourse.mybir as mybir
from concourse.tile import TileContext
from concourse.bass_utils import run_bass_kernel_spmd

EPS = 1e-5
GROUPS = 4
N, C, H, W = 32, 512, 32, 32
HW = H * W                 # 1024
NCORES = 8
NPER = N // NCORES         # 4 images per core
CT = C // 128              # 4 channel tiles
F32 = mybir.dt.float32
BF16 = mybir.dt.bfloat16

_NC_CACHE = None


def _build_nc():
    """Build the (SPMD, per-core) Bass program. Same program on all 8 cores."""
    nc = bacc.Bacc(None, enable_partition_id=False)

    x_d = nc.dram_tensor("x", [NPER, CT, 128, HW], BF16, kind="ExternalInput")
    w_d = nc.dram_tensor("w", [128, CT * CT * 128], BF16, kind="ExternalInput")
    b_d = nc.dram_tensor("b", [128, CT], F32, kind="ExternalInput")
    o_d = nc.dram_tensor("o", [NPER, CT, 128, HW], BF16, kind="ExternalOutput")

    with TileContext(nc) as tc:
        with (
            tc.tile_pool(name="const", bufs=1) as const,
            tc.tile_pool(name="xin", bufs=4) as xin,
            tc.tile_pool(name="act", bufs=3) as actp,
            tc.tile_pool(name="pp", bufs=8, space="PSUM") as pp,
            tc.tile_pool(name="outp", bufs=2) as outp,
        ):
            # Bias (2KB, fp32 - tensor_scalar's vector operand must be f32)
            # then weights, both on the Scalar HWDGE ring.
            bt = const.tile([128, CT], F32)
            nc.scalar.dma_start(bt[:], b_d[:])
            wt = const.tile([128, CT * CT * 128], BF16)
            nc.scalar.dma_start(wt[:], w_d[:])

            # PE warm-up: the HAM clock gate holds the PE at 1.2 GHz until
            # ~3.4us of sustained activity. Burn the DMA-ramp window (first
            # x chunk + relu land ~2us after the PE can start) on dummy
            # matmuls over zeroed scratch - just enough to cover that gap;
            # more would displace the real stream. The dummy PSUM tile
            # shares tag ps0 and is released before image 0 needs the bank.
            wu = const.tile([128, 256], BF16)
            nc.vector.memset(wu[:], 0.0)
            wu_ps = pp.tile([128, 1024], F32, name="wu_ps", tag="ps0", bufs=1)
            for _ in range(11):
                nc.tensor.matmul(
                    wu_ps[:, :256], wu[:, :128], wu[:, :256],
                    start=True, stop=True,
                )

            # Input chunks alternate the two HWDGE rings (Sync + Scalar),
            # all pre-issued; outputs split between the SWDGE queue and the
            # HWDGE rings once input drains.
            xts = []
            for n in range(NPER):
                xt = xin.tile([128, CT * HW], BF16, name=f"xt{n}", tag="xt")
                xts.append(xt)
                for ct in range(CT):
                    eng = nc.sync if ct % 2 == 0 else nc.scalar
                    eng.dma_start(xt[:, ct * HW:(ct + 1) * HW], x_d[n, ct])

            for n in range(NPER):
                xt = xts[n]
                ut = actp.tile([128, CT * HW], BF16)
                # 8 PSUM banks accumulate ct-major, so matmuls start as soon
                # as the first channel tile lands instead of after the last.
                # One [128,1024] (2-bank) tile per d-tile: each matmul writes
                # a single bank, but evacuation runs as one FD=1024 copy.
                pss = [
                    pp.tile([128, 1024], F32, name=f"ps_{n}_{j}", tag=f"ps{j}", bufs=1)
                    for j in range(CT)
                ]
                for ct in range(CT):
                    # relu(x + b) on DVE: bf16 tensor_scalar runs the 4x
                    # packed mode, and keeps ScalarE free for its HWDGE ring
                    nc.vector.tensor_scalar(
                        ut[:, ct * HW:(ct + 1) * HW],
                        xt[:, ct * HW:(ct + 1) * HW],
                        bt[:, ct:ct + 1],
                        0.0,
                        mybir.AluOpType.add,
                        mybir.AluOpType.max,
                    )
                    for dt_ in range(CT):
                        for half in range(2):
                            wcol = (ct * CT + dt_) * 128
                            ucol = ct * HW + half * 512
                            nc.tensor.matmul(
                                pss[dt_][:, half * 512:(half + 1) * 512],
                                wt[:, wcol:wcol + 128],
                                ut[:, ucol:ucol + 512],
                                start=(ct == 0),
                                stop=(ct == CT - 1),
                            )

                last = n == NPER - 1
                ot = outp.tile([128, CT * HW], BF16)
                if not last:
                    # PSUM evacuation (with the fp32->bf16 cast) split
                    # between ACT (dt0/dt2) and DVE (dt1/dt3); stores go on
                    # the engines that are NOT evacuating: SWDGE for the
                    # ACT-evac'd tiles, the Sync HWDGE ring for the rest
                    # (Sync's input triggers drain by ~12us).
                    for dt_ in range(CT):
                        osl = ot[:, dt_ * HW:(dt_ + 1) * HW]
                        if dt_ % 2 == 0:
                            nc.scalar.copy(osl, pss[dt_][:])
                            nc.gpsimd.dma_start(o_d[n, dt_], osl)
                        else:
                            nc.vector.tensor_copy(osl, pss[dt_][:])
                            nc.sync.dma_start(o_d[n, dt_], osl)
                else:
                    # Last image: drain at half-bank granularity so the
                    # first chunks stream out while the last matmuls are
                    # still running; alternate ACT/DVE and spread the
                    # stores across all three DMA paths.
                    st_eng = [nc.sync, nc.scalar, nc.gpsimd]
                    for k in range(2 * CT):
                        dt_, h = divmod(k, 2)
                        osl = ot[:, dt_ * HW + h * 512:dt_ * HW + (h + 1) * 512]
                        psl = pss[dt_][:, h * 512:(h + 1) * 512]
                        if h == 0:
                            nc.scalar.copy(osl, psl)
                        else:
                            nc.vector.tensor_copy(osl, psl)
                        st_eng[k % 3].dma_start(
                            o_d[n, dt_][:, h * 512:(h + 1) * 512], osl
                        )

    nc.finalize()
    return nc


def _prep_inputs(x, gamma, beta, running_mean, running_var, weight, index):
    """Fold BN/shuffle/conv/index into (per-core x shards, weight matrix)."""
    f64 = np.float64
    x = np.asarray(x)
    gamma = np.asarray(gamma).astype(f64)
    beta = np.asarray(beta).astype(f64)
    mean = np.asarray(running_mean).astype(f64)
    var = np.asarray(running_var).astype(f64)
    weight = np.asarray(weight)
    index = np.asarray(index)
    Wc = weight.reshape(C, C // GROUPS).astype(f64)   # (Cout, Cin_per_group)
    idx = index.astype(f64)

    inv = gamma / np.sqrt(var + EPS)                  # > 0
    beta_term = beta - mean * inv
    inv_safe = np.where(inv != 0.0, inv, 1.0)
    bprime = np.where(inv != 0.0, beta_term / inv_safe, 0.0)

    # A[o, c]: conv-after-shuffle as one 512x512 matrix.
    # shuffled channel g*128 + i comes from original channel i*GROUPS + g.
    A = np.zeros((C, C), dtype=f64)
    o = np.arange(C)
    i = np.arange(C // GROUPS)
    src = i[None, :] * GROUPS + (o[:, None] // (C // GROUPS))  # (512, 128)
    A[o[:, None], src] = Wc

    # out[d] = sum_c B[d,c] relu(x_c + bprime_c);  B = (idx^T @ A) * inv
    # Stationary operand is B^T[c, d] = (A^T @ idx) * inv[:, None]
    BT = (A.T @ idx) * inv[:, None]                   # (c, d)

    bf16 = np.dtype(mybir.dt.np(BF16))

    w_host = np.ascontiguousarray(
        BT.reshape(CT, 128, CT, 128).transpose(1, 0, 2, 3).reshape(128, CT * CT * 128)
    ).astype(np.float32).astype(bf16)

    xr = np.ascontiguousarray(
        x.reshape(NCORES, NPER, CT, 128, HW)
    ).astype(bf16)
    b_host = np.ascontiguousarray(
        bprime.astype(np.float32).reshape(CT, 128).T
    )                                                  # (128, CT)
    return [{"x": xr[k], "w": w_host, "b": b_host} for k in range(NCORES)]


def _unpack_output(res):
    out = np.concatenate(
        [np.asarray(res.results[k]["o"]) for k in range(NCORES)], axis=0
    )
    return out.astype(np.float32).reshape(N, C, H, W)


def _run(inputs, trace=False):
    global _NC_CACHE
    if _NC_CACHE is None:
        _NC_CACHE = _build_nc()
    in_maps = _prep_inputs(**inputs)
    res = run_bass_kernel_spmd(_NC_CACHE, in_maps, list(range(NCORES)), trace=trace)
    return _unpack_output(res), res


def kernel(**inputs):
    out, _ = _run(inputs, trace=False)
    return out


# revision 14
# speedup vs baseline: 1.0029x; 1.0029x over previous
"""Trainium2 Bass kernel for nn_CondenseSFR (BN+ReLU+shuffle+grouped1x1conv+reindex).

Algebra: out = einsum('nchw,cd->ndhw', conv(shuffle(relu(bn(x)))), index).
Everything except the ReLU is linear in the channel dimension, and the BN
scale inv = gamma*rsqrt(var+eps) is strictly positive, so
    relu(inv*x + b) = inv * relu(x + b/inv)
and the shuffle + grouped conv + reindex fold into a single dense 512x512
channel matrix applied after the ReLU:
    out[n,d,s] = sum_c B[d,c] * relu(x[n,c,s] + bprime[c])
with B = (index^T @ A) * inv[None,:],  A the shuffle-permuted block-diagonal
conv weight, bprime = (beta - mean*inv)/inv.

Tolerance is 2e-2 so the whole pipeline runs in bf16 (measured end-to-end
rel err ~3.3e-3): x, bias, weights and the stored output are bf16; matmul
accumulation stays fp32 in PSUM. vs the fp32 version this halves HBM
traffic (17.8 -> 8.9 MB/core, the fp32 kernel sat at the ~358 GB/s
HBM-per-core wall) and makes the PE the critical path (~27.5us of N=512
bf16 matmul streaming per core).

Device work per core (4 of 32 images, data-parallel over 8 cores):
  - x image loads as 4 partition-tiles [128, 1024] bf16; the per-channel
    ReLU bias rides in a separate tiny fp32 [128, CT] tensor loaded once
    up front (tensor_scalar's vector operand must be f32)
  - input chunks alternate the two HWDGE rings (Sync+Scalar); outputs
    split between the SWDGE queue and the HWDGE rings as they drain
  - VectorE: relu(x + bias_c) as one fused tensor_scalar(add,max) in
    bf16 (4x packed mode)
  - TensorE: ct-major PSUM accumulation - 8 banks hold all (d-tile, half)
    outputs of one image, so matmuls start when the first channel tile
    lands; bf16 weights get the automatic FWL fast weight load
  - PSUM -> SBUF evacuation casts fp32 -> bf16 (ACT engine, DVE helps on
    the last image) so stores are half-width too
"""

import numpy as np

import concourse.bacc as bacc
import concourse.mybir as mybir
from concourse.tile import TileContext
from concourse.bass_utils import run_bass_kernel_spmd

EPS = 1e-5
GROUPS = 4
N, C, H, W = 32, 512, 32, 32
HW = H * W                 # 1024
NCORES = 8
NPER = N // NCORES         # 4 images per core
CT = C // 128              # 4 channel tiles
F32 = mybir.dt.float32
BF16 = mybir.dt.bfloat16

_NC_CACHE = None


def _build_nc():
    """Build the (SPMD, per-core) Bass program. Same program on all 8 cores."""
    nc = bacc.Bacc(None, enable_partition_id=False)

    x_d = nc.dram_tensor("x", [NPER, CT, 128, HW], BF16, kind="ExternalInput")
    w_d = nc.dram_tensor("w", [128, CT * CT * 128], BF16, kind="ExternalInput")
    b_d = nc.dram_tensor("b", [128, CT], F32, kind="ExternalInput")
    o_d = nc.dram_tensor("o", [NPER, CT, 128, HW], BF16, kind="ExternalOutput")

    with TileContext(nc) as tc:
        with (
            tc.tile_pool(name="const", bufs=1) as const,
            tc.tile_pool(name="xin", bufs=4) as xin,
            tc.tile_pool(name="act", bufs=3) as actp,
            tc.tile_pool(name="pp", bufs=8, space="PSUM") as pp,
            tc.tile_pool(name="outp", bufs=2) as outp,
        ):
            # Bias (2KB, fp32 - tensor_scalar's vector operand must be f32)
            # then weights, both on the Scalar HWDGE ring.
            bt = const.tile([128, CT], F32)
            nc.scalar.dma_start(bt[:], b_d[:])
            wt = const.tile([128, CT * CT * 128], BF16)
            nc.scalar.dma_start(wt[:], w_d[:])

            # PE warm-up: the HAM clock gate holds the PE at 1.2 GHz until
            # ~3.4us of sustained activity. Burn the DMA-ramp window (first
            # x chunk + relu land ~2us after the PE can start) on dummy
            # matmuls over zeroed scratch - just enough to cover that gap;
            # more would displace the real stream. The dummy PSUM tile
            # shares tag ps0 and is released before image 0 needs the bank.
            wu = const.tile([128, 256], BF16)
            nc.vector.memset(wu[:], 0.0)
            wu_ps = pp.tile([128, 1024], F32, name="wu_ps", tag="ps0", bufs=1)
            for _ in range(24):
                nc.tensor.matmul(
                    wu_ps[:, :256], wu[:, :128], wu[:, :256],
                    start=True, stop=True,
                )

            # Input chunks alternate the two HWDGE rings (Sync + Scalar),
            # all pre-issued; outputs split between the SWDGE queue and the
            # HWDGE rings once input drains.
            xts = []
            for n in range(NPER):
                xt = xin.tile([128, CT * HW], BF16, name=f"xt{n}", tag="xt")
                xts.append(xt)
                for ct in range(CT):
                    eng = nc.sync if ct % 2 == 0 else nc.scalar
                    eng.dma_start(xt[:, ct * HW:(ct + 1) * HW], x_d[n, ct])

            for n in range(NPER):
                xt = xts[n]
                ut = actp.tile([128, CT * HW], BF16)
                # 8 PSUM banks accumulate ct-major, so matmuls start as soon
                # as the first channel tile lands instead of after the last.
                # One [128,1024] (2-bank) tile per d-tile: each matmul writes
                # a single bank, but evacuation runs as one FD=1024 copy.
                pss = [
                    pp.tile([128, 1024], F32, name=f"ps_{n}_{j}", tag=f"ps{j}", bufs=1)
                    for j in range(CT)
                ]
                for ct in range(CT):
                    # relu(x + b) on DVE: bf16 tensor_scalar runs the 4x
                    # packed mode, and keeps ScalarE free for its HWDGE ring
                    nc.vector.tensor_scalar(
                        ut[:, ct * HW:(ct + 1) * HW],
                        xt[:, ct * HW:(ct + 1) * HW],
                        bt[:, ct:ct + 1],
                        0.0,
                        mybir.AluOpType.add,
                        mybir.AluOpType.max,
                    )
                    for dt_ in range(CT):
                        for half in range(2):
                            wcol = (ct * CT + dt_) * 128
                            ucol = ct * HW + half * 512
                            nc.tensor.matmul(
                                pss[dt_][:, half * 512:(half + 1) * 512],
                                wt[:, wcol:wcol + 128],
                                ut[:, ucol:ucol + 512],
                                start=(ct == 0),
                                stop=(ct == CT - 1),
                            )

                last = n == NPER - 1
                ot = outp.tile([128, CT * HW], BF16)
                for dt_ in range(CT):
                    osl = ot[:, dt_ * HW:(dt_ + 1) * HW]
                    # PSUM evacuation (with the fp32->bf16 cast) on ACT.
                    # Mid-stream the DVE must stay relu-only: engine queues
                    # are in-order, so evac work on the DVE would head-of-
                    # line-block the next image's relus behind this image's
                    # last matmuls. On the last image there are no later
                    # relus, so the drain splits ACT/DVE by whole psum
                    # tiles (whole, because the Tile tracker serializes
                    # cross-engine access at tile granularity).
                    if last and dt_ % 2 == 1:
                        nc.vector.tensor_copy(osl, pss[dt_][:])
                    else:
                        nc.scalar.copy(osl, pss[dt_][:])
                    # store triggers live on Sync/SWDGE so they never queue
                    # behind (or in front of) ACT's evacuations
                    if dt_ % 2 == 1:
                        nc.sync.dma_start(o_d[n, dt_], osl)
                    else:
                        nc.gpsimd.dma_start(o_d[n, dt_], osl)

    nc.finalize()
    return nc


def _prep_inputs(x, gamma, beta, running_mean, running_var, weight, index):
    """Fold BN/shuffle/conv/index into (per-core x shards, weight matrix)."""
    f64 = np.float64
    x = np.asarray(x)
    gamma = np.asarray(gamma).astype(f64)
    beta = np.asarray(beta).astype(f64)
    mean = np.asarray(running_mean).astype(f64)
    var = np.asarray(running_var).astype(f64)
    weight = np.asarray(weight)
    index = np.asarray(index)
    Wc = weight.reshape(C, C // GROUPS).astype(f64)   # (Cout, Cin_per_group)
    idx = index.astype(f64)

    inv = gamma / np.sqrt(var + EPS)                  # > 0
    beta_term = beta - mean * inv
    inv_safe = np.where(inv != 0.0, inv, 1.0)
    bprime = np.where(inv != 0.0, beta_term / inv_safe, 0.0)

    # A[o, c]: conv-after-shuffle as one 512x512 matrix.
    # shuffled channel g*128 + i comes from original channel i*GROUPS + g.
    A = np.zeros((C, C), dtype=f64)
    o = np.arange(C)
    i = np.arange(C // GROUPS)
    src = i[None, :] * GROUPS + (o[:, None] // (C // GROUPS))  # (512, 128)
    A[o[:, None], src] = Wc

    # out[d] = sum_c B[d,c] relu(x_c + bprime_c);  B = (idx^T @ A) * inv
    # Stationary operand is B^T[c, d] = (A^T @ idx) * inv[:, None]
    BT = (A.T @ idx) * inv[:, None]                   # (c, d)

    bf16 = np.dtype(mybir.dt.np(BF16))

    w_host = np.ascontiguousarray(
        BT.reshape(CT, 128, CT, 128).transpose(1, 0, 2, 3).reshape(128, CT * CT * 128)
    ).astype(np.float32).astype(bf16)

    xr = np.ascontiguousarray(
        x.reshape(NCORES, NPER, CT, 128, HW)
    ).astype(bf16)
    b_host = np.ascontiguousarray(
        bprime.astype(np.float32).reshape(CT, 128).T
    )                                                  # (128, CT)
    return [{"x": xr[k], "w": w_host, "b": b_host} for k in range(NCORES)]


def _unpack_output(res):
    out = np.concatenate(
        [np.asarray(res.results[k]["o"]) for k in range(NCORES)], axis=0
    )
    return out.astype(np.float32).reshape(N, C, H, W)


def _run(inputs, trace=False):
    global _NC_CACHE
    if _NC_CACHE is None:
        _NC_CACHE = _build_nc()
    in_maps = _prep_inputs(**inputs)
    res = run_bass_kernel_spmd(_NC_CACHE, in_maps, list(range(NCORES)), trace=trace)
    return _unpack_output(res), res


def kernel(**inputs):
    out, _ = _run(inputs, trace=False)
    return out
